# revision 4
# baseline (speedup 1.0000x reference)
"""LinearOffsetLayer Trainium2 kernel (8 NeuronCores, tensor-parallel on out_features).

Math:  A[o,i] = sum_d theta_d[d] * P_A[o,d,i] + theta0_A[o,i]
       b[o]   = theta_d @ P_b + theta0_b
       out    = input @ A.T + b                          # [4096, 1024]

Sharding: out_features (o) split 8 ways -> 128 o per core.  Each core gets its
P_A / theta0_A / P_b / theta0_b shard; input (pre-transposed on host to
[in_f, n]) and theta_d are replicated.  Each core computes out_T shard
[128, 4096]; host concatenates and transposes back.

Per-core dataflow:
  1. einsum: for each o (128): DMA P_A[o] ([d=128, i=1024], 256 KB contiguous),
     then 8 matmuls with the [d,128] slice as the *stationary* operand and
     theta_d [d,1] as the N=1 moving operand -> A_T column [i_local, 1] in
     PSUM.  This streams P_A through the PE at LDWEIGHTS rate (~612 GB/s),
     fully hidden under the ~358 GB/s HBM DMA stream.
  2. A_T[k] = PSUM + theta0_A_T[k]  (DVE add, evacuates PSUM).
  3. main matmul: out_T[:, nb] = sum_k A_T[k].T @ x_T[k, nb], k-inner PSUM
     accumulation, bias fused into the PSUM->SBUF eviction.
"""

import numpy as np

import concourse.bass as bass
import concourse.bacc as bacc
import concourse.mybir as mybir
import concourse.tile as tile
from concourse.bass_utils import run_bass_kernel_spmd

P = 128          # partitions / d / per-core o-shard
IN_F = 1024
OUT_F = 1024
NTOK = 4096
NCORES = 8
KB = IN_F // P   # 8 k-blocks of the contraction dim
FD = 512         # fp32 moving-operand max free dim
NB = NTOK // FD  # 8 n-blocks
F32 = mybir.dt.float32

_CACHE = {}


def _build():
    nc = bacc.Bacc("TRN2", target_bir_lowering=False, debug=False,
                   num_devices=NCORES)

    x_d = nc.dram_tensor("xT", [IN_F, NTOK], F32, kind="ExternalInput")
    th_d = nc.dram_tensor("theta", [P, 1], F32, kind="ExternalInput")
    pa_d = nc.dram_tensor("pa", [P, P, IN_F], F32, kind="ExternalInput")
    t0a_d = nc.dram_tensor("t0aT", [IN_F, P], F32, kind="ExternalInput")
    pb_d = nc.dram_tensor("pb", [P, P], F32, kind="ExternalInput")
    t0b_d = nc.dram_tensor("t0b", [P, 1], F32, kind="ExternalInput")
    out_d = nc.dram_tensor("out", [P, NTOK], F32, kind="ExternalOutput")

    with tile.TileContext(nc) as tc:
        from contextlib import ExitStack
        with ExitStack() as ctx:
            consts = ctx.enter_context(tc.tile_pool(name="consts", bufs=1))
            inp_pool = ctx.enter_context(tc.tile_pool(name="inp", bufs=KB))
            pa_pool = ctx.enter_context(tc.tile_pool(name="pa", bufs=4))
            asb_pool = ctx.enter_context(tc.tile_pool(name="asb", bufs=1))
            ps_e = ctx.enter_context(
                tc.tile_pool(name="ps_e", bufs=2, space="PSUM"))
            ps_b = ctx.enter_context(
                tc.tile_pool(name="ps_b", bufs=1, space="PSUM"))
            ps_o = ctx.enter_context(
                tc.tile_pool(name="ps_o", bufs=3, space="PSUM"))
            outsb = ctx.enter_context(tc.tile_pool(name="outsb", bufs=3))

            th_sb = consts.tile([P, 1], F32)
            nc.sync.dma_start(th_sb[:], th_d[:, :])
            pb_sb = consts.tile([P, P], F32)
            nc.sync.dma_start(pb_sb[:], pb_d[:, :])
            t0b_sb = consts.tile([P, 1], F32)
            nc.sync.dma_start(t0b_sb[:], t0b_d[:, :])
            t0a_sb = consts.tile([P, IN_F], F32)
            for k in range(KB):
                nc.sync.dma_start(t0a_sb[:, k * P:(k + 1) * P],
                                  t0a_d[k * P:(k + 1) * P, :])
            b_sb = consts.tile([P, 1], F32)

            # resident input (transposed) tiles: x_sb[k] = x_T[k*128:(k+1)*128, :]
            x_sb = []
            for k in range(KB):
                xt = inp_pool.tile([P, NTOK], F32)
                nc.sync.dma_start(xt[:], x_d[k * P:(k + 1) * P, :])
                x_sb.append(xt)

            # bias: b = P_b.T @ theta + theta0_b     [o, 1]
            bp = ps_b.tile([P, 1], F32)
            nc.tensor.matmul(bp[:], lhsT=pb_sb[:], rhs=th_sb[:],
                             start=True, stop=True)
            nc.vector.tensor_add(b_sb[:], bp[:], t0b_sb[:])

            # einsum: A_T columns
            psum_e = [ps_e.tile([P, FD], F32, name=f"psum_e{j}", tag="psum_e")
                      for j in range(2)]
            for o in range(P):
                pa_t = pa_pool.tile([P, IN_F], F32)
                nc.sync.dma_start(pa_t[:], pa_d[o, :, :])
                for k in range(KB):
                    col = (k % 4) * P + o
                    nc.tensor.matmul(
                        psum_e[k // 4][:, col:col + 1],
                        lhsT=pa_t[:, k * P:(k + 1) * P],
                        rhs=th_sb[:],
                        start=True, stop=True)

            a_sb = asb_pool.tile([P, IN_F], F32)
            for k in range(KB):
                nc.vector.tensor_add(
                    a_sb[:, k * P:(k + 1) * P],
                    psum_e[k // 4][:, (k % 4) * P:(k % 4 + 1) * P],
                    t0a_sb[:, k * P:(k + 1) * P])

            # main matmul: out_T[:, nb] = sum_k A_T[k].T @ x_T[k][:, nb] ; + b
            for nb in range(NB):
                po = ps_o.tile([P, FD], F32)
                for k in range(KB):
                    nc.tensor.matmul(
                        po[:],
                        lhsT=a_sb[:, k * P:(k + 1) * P],
                        rhs=x_sb[k][:, nb * FD:(nb + 1) * FD],
                        start=(k == 0), stop=(k == KB - 1))
                ot = outsb.tile([P, FD], F32)
                nc.vector.tensor_scalar_add(ot[:], po[:], b_sb[:, 0:1])
                nc.sync.dma_start(out_d[:, nb * FD:(nb + 1) * FD], ot[:])

    nc.compile()
    return nc


def _in_maps(inputs):
    x = np.asarray(inputs["input"], dtype=np.float32)
    theta_d = np.asarray(inputs["theta_d"], dtype=np.float32)
    theta0_A = np.asarray(inputs["theta0_A"], dtype=np.float32)
    P_A = np.asarray(inputs["P_A"], dtype=np.float32)
    theta0_b = np.asarray(inputs["theta0_b"], dtype=np.float32)
    P_b = np.asarray(inputs["P_b"], dtype=np.float32)

    xT = np.ascontiguousarray(x.T)                    # [in_f, n]
    th = np.ascontiguousarray(theta_d.reshape(P, 1))
    t0aT = np.ascontiguousarray(theta0_A.T)           # [in_f, out_f]

    maps = []
    for c in range(NCORES):
        o0 = c * P
        maps.append({
            "xT": xT,
            "theta": th,
            "pa": np.ascontiguousarray(P_A[o0:o0 + P]),
            "t0aT": np.ascontiguousarray(t0aT[:, o0:o0 + P]),
            "pb": np.ascontiguousarray(P_b[:, o0:o0 + P]),
            "t0b": np.ascontiguousarray(theta0_b[o0:o0 + P].reshape(P, 1)),
        })
    return maps


def run(inputs, trace=False):
    """Returns (output [4096,1024] f32, exec_time_ns or None)."""
    if "nc" not in _CACHE:
        _CACHE["nc"] = _build()
    nc = _CACHE["nc"]
    res = run_bass_kernel_spmd(nc, _in_maps(inputs),
                               core_ids=list(range(NCORES)), trace=trace)
    shards = [res.results[c]["out"] for c in range(NCORES)]   # [128, 4096] each
    outT = np.concatenate(shards, axis=0)                     # [out_f, n]
    return np.ascontiguousarray(outT.T), res.exec_time_ns


def kernel(**inputs):
    out, _ = run(inputs, trace=False)
    return out
